# revision 28
# baseline (speedup 1.0000x reference)
"""Balanced dice loss (histogram binning) on 8 Trainium2 NeuronCores.

Math: with t ∈ {0,1} and p = sigmoid(x), the loss needs four global sums:
    S_t   = Σ t            (the bincount)
    S_pt  = Σ p·t
    S_pp  = Σ p²
    S_ppt = Σ p²·t
Then with c1 = S_t, c0 = N − c1, w0 = 1/(c0+s)², w1 = 1/(c1+s)²:
    intersection = w1·S_pt
    denominator  = w0·(S_pp − S_ppt) + w1·(S_ppt + c1)
    dice = 1 − (2·I + s)/(D + s)

Device kernel (data-parallel over 8 cores, batch-sharded), per [128,F] tile:
    ACT : p = sigmoid(x) (bf16); tb = copy(t) int32→bf16 with row-accum
          → S_t (one pass converts dtype AND takes the bincount);
          square(p) on the first FH columns with row-accum → S_pp part 1
    DVE : u = p·tb, w = u·p (= p²·t) in bf16 2× perf mode; sq = p·p on
          the remaining columns + f32 row-reduce → S_pp part 2
    PE  : ones[128,128] @ 512-col chunks of u and w → two PSUM column-sum
          accumulation chains (S_pt, S_ppt), alternating banks per chunk
The split keeps every engine under the ~82µs HBM stream (32 MB/core at
~410 GB/s measured) in EVERY PE clock-gate (HAM) state: 128 matmuls fit
the stream even fully cold at 1.2 GHz, removing that source of
run-to-run spread. Tiling is 15×2048 + 4×512 columns: the short
trailing tiles shrink the serial drain chain after the last DMA lands
(small tiles put all of S_pp on the DVE side). p/tb and u/w share one
pool tile each to cut pool-bookkeeping semaphores (end-of-kernel
teardown). Partials are DMA'd out; host reduces in float64 and
finishes the scalar math.
"""

import numpy as np

import concourse.bacc as bacc
import concourse.mybir as mybir
from concourse.bass_utils import run_bass_kernel_spmd
from concourse.tile import TileContext

N_CORES = 8
P = 128
TOTAL = 32 * 1024 * 1024  # elements in the full problem
PER_CORE = TOTAL // N_CORES  # 4,194,304
FREE = PER_CORE // P  # 32,768 f32 per partition
F = 2048  # big tile free-dim
FS = 512  # small trailing tile free-dim
NBIG = 15
NSMALL = (FREE - NBIG * F) // FS  # 4
SIZES = [F] * NBIG + [FS] * NSMALL
MMN = 512  # matmul moving free-dim (one PSUM bank; ISA max)
FH = F // 4  # big-tile S_pp split: [:FH] on ACT, [FH:] on DVE
SMOOTH = 1e-05

# s_pp column layout: 2 per big tile (ACT part, DVE part), 1 per small tile
NPP = 2 * NBIG + NSMALL
NTT = NBIG + NSMALL

_nc_cache = None


def _build_bass():
    nc = bacc.Bacc(None, target_bir_lowering=False)
    x = nc.dram_tensor("input", [P, FREE], mybir.dt.float32, kind="ExternalInput")
    t = nc.dram_tensor("target", [P, FREE], mybir.dt.int32, kind="ExternalInput")
    o_sums = nc.dram_tensor(
        "o_sums", [1, 4 * MMN], mybir.dt.float32, kind="ExternalOutput"
    )
    o_st = nc.dram_tensor("o_st", [P, NTT], mybir.dt.float32, kind="ExternalOutput")
    o_pp = nc.dram_tensor("o_pp", [P, NPP], mybir.dt.float32, kind="ExternalOutput")

    G = FREE // MMN  # total matmul chunks per chain (64)

    with TileContext(nc) as tc:
        with (
            tc.tile_pool(name="work", bufs=2) as pool,
            tc.tile_pool(name="stats", bufs=1) as spool,
            tc.tile_pool(name="ps", bufs=1, space="PSUM") as psum,
        ):
            s_t = spool.tile([P, NTT], mybir.dt.float32)
            s_pp = spool.tile([P, NPP], mybir.dt.float32)
            ones = spool.tile([P, P], mybir.dt.bfloat16, tag="ones")
            junk = spool.tile([P, FH], mybir.dt.bfloat16, tag="junk")
            ps_pt_a = psum.tile([P, MMN], mybir.dt.float32, tag="ps_pt_a")
            ps_pt_b = psum.tile([P, MMN], mybir.dt.float32, tag="ps_pt_b")
            ps_ppt_a = psum.tile([P, MMN], mybir.dt.float32, tag="ps_ppt_a")
            ps_ppt_b = psum.tile([P, MMN], mybir.dt.float32, tag="ps_ppt_b")

            # emit the first tile's loads before the ones-memset so the
            # sync queue reaches them as early as possible
            xts, tts = [], []
            off = 0
            for i, Fi in enumerate(SIZES):
                big = Fi == F
                xt = pool.tile(
                    [P, Fi], mybir.dt.float32, tag="xt" if big else "xts", bufs=6
                )
                tt = pool.tile(
                    [P, Fi], mybir.dt.int32, tag="tt" if big else "tts", bufs=6
                )
                nc.sync.dma_start(xt[:], x[:, off : off + Fi])
                nc.sync.dma_start(tt[:], t[:, off : off + Fi])
                xts.append(xt)
                tts.append(tt)
                off += Fi
                if i == 0:
                    nc.any.memset(ones, 1.0)

            g = 0  # global chunk counter (both chains advance together)
            ppcol = 0
            for i, Fi in enumerate(SIZES):
                big = Fi == F
                xt, tt = xts[i], tts[i]
                ptb = pool.tile(
                    [P, 2 * Fi], mybir.dt.bfloat16, tag="ptb" if big else "ptbs",
                    bufs=3,
                )
                uw = pool.tile(
                    [P, 2 * Fi], mybir.dt.bfloat16, tag="uw" if big else "uws"
                )
                sq = pool.tile(
                    [P, Fi - FH if big else Fi], mybir.dt.bfloat16,
                    tag="sq" if big else "sqs",
                )
                p_, tb = ptb[:, :Fi], ptb[:, Fi:]
                u, w = uw[:, :Fi], uw[:, Fi:]

                # p = sigmoid(x); tb = float(t) with S_t row-accum   [ACT]
                nc.scalar.activation(
                    p_, xt[:], mybir.ActivationFunctionType.Sigmoid
                )
                nc.scalar.activation(
                    tb,
                    tt[:],
                    mybir.ActivationFunctionType.Copy,
                    accum_out=s_t[:, i : i + 1],
                )
                # u = p·t, w = u·p = p²t (bf16, 2x mode)             [DVE]
                nc.vector.tensor_tensor(
                    out=u, in0=p_, in1=tb, op=mybir.AluOpType.mult
                )
                nc.vector.tensor_tensor(
                    out=w, in0=u, in1=p_, op=mybir.AluOpType.mult
                )
                # column-sum accumulation chains; banks alternate per
                # global chunk to pipeline the PSUM RMW              [PE]
                nch = Fi // MMN
                for cn, (base, banks) in enumerate(
                    ((0, (ps_pt_a, ps_pt_b)), (Fi, (ps_ppt_a, ps_ppt_b)))
                ):
                    for jj in range(nch):
                        gj = g + jj
                        nc.tensor.matmul(
                            banks[gj % 2][:],
                            ones[:],
                            uw[:, base + jj * MMN : base + (jj + 1) * MMN],
                            start=(gj < 2),
                            stop=(gj >= G - 2),
                        )
                g += nch
                # S_pp: big tiles split ACT square / DVE square+reduce;
                # small trailing tiles go DVE-only (short drain chain)
                if big:
                    nc.scalar.activation(
                        junk[:],
                        p_[:, :FH],
                        mybir.ActivationFunctionType.Square,
                        accum_out=s_pp[:, ppcol : ppcol + 1],
                    )
                    nc.vector.tensor_tensor(
                        out=sq[:], in0=p_[:, FH:], in1=p_[:, FH:],
                        op=mybir.AluOpType.mult,
                    )
                    nc.vector.tensor_reduce(
                        s_pp[:, ppcol + 1 : ppcol + 2],
                        sq[:],
                        axis=mybir.AxisListType.X,
                        op=mybir.AluOpType.add,
                    )
                    ppcol += 2
                else:
                    nc.vector.tensor_tensor(
                        out=sq[:], in0=p_, in1=p_, op=mybir.AluOpType.mult
                    )
                    nc.vector.tensor_reduce(
                        s_pp[:, ppcol : ppcol + 1],
                        sq[:],
                        axis=mybir.AxisListType.X,
                        op=mybir.AluOpType.add,
                    )
                    ppcol += 1

            fin = spool.tile([1, 4 * MMN], mybir.dt.float32, tag="fin")
            for k, ps in enumerate((ps_pt_a, ps_pt_b, ps_ppt_a, ps_ppt_b)):
                dst = fin[:, k * MMN : (k + 1) * MMN]
                if k % 2 == 0:
                    nc.vector.tensor_copy(dst, ps[0:1, :])
                else:
                    nc.scalar.copy(dst, ps[0:1, :])
            nc.sync.dma_start(o_sums[:], fin[:])
            nc.sync.dma_start(o_st[:], s_t[:])
            nc.sync.dma_start(o_pp[:], s_pp[:])
    nc.finalize()
    return nc


def _get_nc():
    global _nc_cache
    if _nc_cache is None:
        _nc_cache = _build_bass()
    return _nc_cache


def kernel(input, target, _trace=False):
    x = np.ascontiguousarray(np.asarray(input, dtype=np.float32)).reshape(
        N_CORES, P, FREE
    )
    t = np.ascontiguousarray(np.asarray(target, dtype=np.int32)).reshape(
        N_CORES, P, FREE
    )
    in_maps = [{"input": x[i], "target": t[i]} for i in range(N_CORES)]

    nc = _get_nc()
    res = run_bass_kernel_spmd(
        nc, in_maps, core_ids=list(range(N_CORES)), trace=_trace
    )
    kernel.last_results = res

    s_pt = s_ppt = s_pp = s_t = 0.0
    for r in res.results:
        sums = r["o_sums"].astype(np.float64)
        s_pt += float(sums[0, 0 : 2 * MMN].sum())
        s_ppt += float(sums[0, 2 * MMN :].sum())
        s_pp += float(r["o_pp"].astype(np.float64).sum())
        s_t += float(r["o_st"].astype(np.float64).sum())

    c1 = float(s_t)
    c0 = float(TOTAL - s_t)
    w0 = 1.0 / (c0 + SMOOTH) ** 2
    w1 = 1.0 / (c1 + SMOOTH) ** 2
    intersection = w1 * s_pt
    denominator = w0 * (s_pp - s_ppt) + w1 * (s_ppt + c1)
    dice = 1.0 - (2.0 * intersection + SMOOTH) / (denominator + SMOOTH)
    return np.asarray(dice, dtype=np.float32)


# revision 29
# speedup vs baseline: 1.1548x; 1.1548x over previous
"""Balanced dice loss (histogram binning) on 8 Trainium2 NeuronCores.

Math: with t ∈ {0,1} and p = sigmoid(x), the loss needs four global sums:
    S_t   = Σ t            (the bincount)
    S_pt  = Σ p·t
    S_pp  = Σ p²
    S_ppt = Σ p²·t
Then with c1 = S_t, c0 = N − c1, w0 = 1/(c0+s)², w1 = 1/(c1+s)²:
    intersection = w1·S_pt
    denominator  = w0·(S_pp − S_ppt) + w1·(S_ppt + c1)
    dice = 1 − (2·I + s)/(D + s)

Device kernel (data-parallel over 8 cores, batch-sharded), per [128,F] tile:
    ACT : p = sigmoid(x) (bf16); tb = copy(t) int32→bf16 with row-accum
          → S_t (one pass converts dtype AND takes the bincount);
          square(p) on the first FH columns with row-accum → S_pp part 1
    DVE : u = p·tb, w = u·p (= p²·t) in bf16 2× perf mode; sq = p·p on
          the remaining columns + f32 row-reduce → S_pp part 2
    PE  : ones[128,128] @ 512-col chunks of u and w → two PSUM column-sum
          accumulation chains (S_pt, S_ppt), each alternating two banks
          to pipeline the PSUM RMW
The split is sized so every engine stays under the ~82µs HBM stream
(32 MB/core at ~410 GB/s measured) in EVERY PE clock-gate (HAM) state:
128 matmuls fit the stream even fully cold at 1.2 GHz, which removes the
throttle-dependent run-to-run spread seen with a third matmul chain.
Partials are DMA'd out; host reduces in float64 and finishes the scalar
math.
"""

import numpy as np

import concourse.bacc as bacc
import concourse.mybir as mybir
from concourse.bass_utils import run_bass_kernel_spmd
from concourse.tile import TileContext

N_CORES = 8
P = 128
TOTAL = 32 * 1024 * 1024  # elements in the full problem
PER_CORE = TOTAL // N_CORES  # 4,194,304
FREE = PER_CORE // P  # 32,768 f32 per partition
F = 2048  # tile free-dim
NT = FREE // F  # 16 tiles per core
MMN = 512  # matmul moving free-dim (one PSUM bank; ISA max)
NCH = F // MMN  # matmul chunks per tile
FH = F // 4  # S_pp split point: [:FH] on ACT, [FH:] on DVE
SMOOTH = 1e-05

_nc_cache = None


def _build_bass():
    nc = bacc.Bacc(None, target_bir_lowering=False)
    x = nc.dram_tensor("input", [P, FREE], mybir.dt.float32, kind="ExternalInput")
    t = nc.dram_tensor("target", [P, FREE], mybir.dt.int32, kind="ExternalInput")
    o_sums = nc.dram_tensor(
        "o_sums", [1, 4 * MMN], mybir.dt.float32, kind="ExternalOutput"
    )
    o_st = nc.dram_tensor("o_st", [P, NT], mybir.dt.float32, kind="ExternalOutput")
    o_pp = nc.dram_tensor("o_pp", [P, 2 * NT], mybir.dt.float32, kind="ExternalOutput")

    with TileContext(nc) as tc:
        with (
            tc.tile_pool(name="work", bufs=2) as pool,
            tc.tile_pool(name="stats", bufs=1) as spool,
            tc.tile_pool(name="ps", bufs=1, space="PSUM") as psum,
        ):
            s_t = spool.tile([P, NT], mybir.dt.float32)
            s_pp = spool.tile([P, 2 * NT], mybir.dt.float32)
            ones = spool.tile([P, P], mybir.dt.bfloat16, tag="ones")
            junk = spool.tile([P, FH], mybir.dt.bfloat16, tag="junk")
            ps_pt_a = psum.tile([P, MMN], mybir.dt.float32, tag="ps_pt_a")
            ps_pt_b = psum.tile([P, MMN], mybir.dt.float32, tag="ps_pt_b")
            ps_ppt_a = psum.tile([P, MMN], mybir.dt.float32, tag="ps_ppt_a")
            ps_ppt_b = psum.tile([P, MMN], mybir.dt.float32, tag="ps_ppt_b")

            # emit the first tile's loads before the ones-memset so the
            # sync queue reaches them as early as possible
            xts, tts = [], []
            for i in range(NT):
                xt = pool.tile([P, F], mybir.dt.float32, tag="xt", bufs=6)
                tt = pool.tile([P, F], mybir.dt.int32, tag="tt", bufs=6)
                nc.sync.dma_start(xt[:], x[:, i * F : (i + 1) * F])
                nc.sync.dma_start(tt[:], t[:, i * F : (i + 1) * F])
                xts.append(xt)
                tts.append(tt)
                if i == 0:
                    nc.any.memset(ones, 1.0)

            for i in range(NT):
                xt, tt = xts[i], tts[i]
                p_ = pool.tile([P, F], mybir.dt.bfloat16, tag="p", bufs=3)
                tb = pool.tile([P, F], mybir.dt.bfloat16, tag="tb", bufs=3)
                u = pool.tile([P, F], mybir.dt.bfloat16, tag="u")
                w = pool.tile([P, F], mybir.dt.bfloat16, tag="w")
                sq = pool.tile([P, F - FH], mybir.dt.bfloat16, tag="sq")

                # p = sigmoid(x); tb = float(t) with S_t row-accum   [ACT]
                nc.scalar.activation(
                    p_[:], xt[:], mybir.ActivationFunctionType.Sigmoid
                )
                nc.scalar.activation(
                    tb[:],
                    tt[:],
                    mybir.ActivationFunctionType.Copy,
                    accum_out=s_t[:, i : i + 1],
                )
                # u = p·t, w = u·p = p²t (bf16, 2x mode)             [DVE]
                nc.vector.tensor_tensor(
                    out=u[:], in0=p_[:], in1=tb[:], op=mybir.AluOpType.mult
                )
                nc.vector.tensor_tensor(
                    out=w[:], in0=u[:], in1=p_[:], op=mybir.AluOpType.mult
                )
                # S_pp: [:FH] via ACT square row-accum …             [ACT]
                nc.scalar.activation(
                    junk[:],
                    p_[:, :FH],
                    mybir.ActivationFunctionType.Square,
                    accum_out=s_pp[:, 2 * i : 2 * i + 1],
                )
                # … and [FH:] via DVE square + f32 row-reduce        [DVE]
                nc.vector.tensor_tensor(
                    out=sq[:], in0=p_[:, FH:], in1=p_[:, FH:], op=mybir.AluOpType.mult
                )
                nc.vector.tensor_reduce(
                    s_pp[:, 2 * i + 1 : 2 * i + 2],
                    sq[:],
                    axis=mybir.AxisListType.X,
                    op=mybir.AluOpType.add,
                )
                # column-sum accumulation chains; each chain
                # alternates two PSUM banks to pipeline the RMW     [PE]
                for s_, banks in (
                    (u, (ps_pt_a, ps_pt_b)),
                    (w, (ps_ppt_a, ps_ppt_b)),
                ):
                    for j in range(NCH):
                        nc.tensor.matmul(
                            banks[j % 2][:],
                            ones[:],
                            s_[:, j * MMN : (j + 1) * MMN],
                            start=(i == 0 and j < 2),
                            stop=(i == NT - 1 and j >= NCH - 2),
                        )

            fin = spool.tile([1, 4 * MMN], mybir.dt.float32, tag="fin")
            for k, ps in enumerate((ps_pt_a, ps_pt_b, ps_ppt_a, ps_ppt_b)):
                dst = fin[:, k * MMN : (k + 1) * MMN]
                if k % 2 == 0:
                    nc.vector.tensor_copy(dst, ps[0:1, :])
                else:
                    nc.scalar.copy(dst, ps[0:1, :])
            nc.sync.dma_start(o_sums[:], fin[:])
            nc.sync.dma_start(o_st[:], s_t[:])
            nc.sync.dma_start(o_pp[:], s_pp[:])
    nc.finalize()
    return nc


def _get_nc():
    global _nc_cache
    if _nc_cache is None:
        _nc_cache = _build_bass()
    return _nc_cache


def kernel(input, target, _trace=False):
    x = np.ascontiguousarray(np.asarray(input, dtype=np.float32)).reshape(
        N_CORES, P, FREE
    )
    t = np.ascontiguousarray(np.asarray(target, dtype=np.int32)).reshape(
        N_CORES, P, FREE
    )
    in_maps = [{"input": x[i], "target": t[i]} for i in range(N_CORES)]

    nc = _get_nc()
    res = run_bass_kernel_spmd(
        nc, in_maps, core_ids=list(range(N_CORES)), trace=_trace
    )
    kernel.last_results = res

    s_pt = s_ppt = s_pp = s_t = 0.0
    for r in res.results:
        sums = r["o_sums"].astype(np.float64)
        s_pt += float(sums[0, 0 : 2 * MMN].sum())
        s_ppt += float(sums[0, 2 * MMN :].sum())
        s_pp += float(r["o_pp"].astype(np.float64).sum())
        s_t += float(r["o_st"].astype(np.float64).sum())

    c1 = float(s_t)
    c0 = float(TOTAL - s_t)
    w0 = 1.0 / (c0 + SMOOTH) ** 2
    w1 = 1.0 / (c1 + SMOOTH) ** 2
    intersection = w1 * s_pt
    denominator = w0 * (s_pp - s_ppt) + w1 * (s_ppt + c1)
    dice = 1.0 - (2.0 * intersection + SMOOTH) / (denominator + SMOOTH)
    return np.asarray(dice, dtype=np.float32)


# revision 31
# speedup vs baseline: 1.1624x; 1.0066x over previous
"""Balanced dice loss (histogram binning) on 8 Trainium2 NeuronCores.

Math: with t ∈ {0,1} and p = sigmoid(x), the loss needs four global sums:
    S_t   = Σ t            (the bincount)
    S_pt  = Σ p·t
    S_pp  = Σ p²
    S_ppt = Σ p²·t
Then with c1 = S_t, c0 = N − c1, w0 = 1/(c0+s)², w1 = 1/(c1+s)²:
    intersection = w1·S_pt
    denominator  = w0·(S_pp − S_ppt) + w1·(S_ppt + c1)
    dice = 1 − (2·I + s)/(D + s)

Device kernel (data-parallel over 8 cores, batch-sharded), per [128,F] tile:
    ACT : p = sigmoid(x) (bf16); tb = copy(t) int32→bf16 with row-accum
          → S_t (one pass does the dtype conversion AND the bincount)
    DVE : u = p·tb, w = u·p (= p²·t), sq = p·p — all-bf16 2× perf mode
    PE  : ones[128,128] @ 512-col chunks of u, w, sq → three PSUM
          column-sum accumulation chains (S_pt, S_ppt, S_pp), each
          alternating two banks to pipeline the RMW
Partials are DMA'd out; host reduces in float64.
"""

import numpy as np

import concourse.bacc as bacc
import concourse.mybir as mybir
from concourse.bass_utils import run_bass_kernel_spmd
from concourse.tile import TileContext

N_CORES = 8
P = 128
TOTAL = 32 * 1024 * 1024  # elements in the full problem
PER_CORE = TOTAL // N_CORES  # 4,194,304
FREE = PER_CORE // P  # 32,768 f32 per partition
F = 2048  # tile free-dim
NT = FREE // F  # 16 tiles per core
MMN = 512  # matmul moving free-dim (one PSUM bank; ISA max)
NCH = F // MMN  # matmul chunks per tile
SMOOTH = 1e-05

_nc_cache = None


def _build_bass():
    nc = bacc.Bacc(None, target_bir_lowering=False)
    x = nc.dram_tensor("input", [P, FREE], mybir.dt.float32, kind="ExternalInput")
    t = nc.dram_tensor("target", [P, FREE], mybir.dt.int32, kind="ExternalInput")
    o_sums = nc.dram_tensor(
        "o_sums", [1, 6 * MMN], mybir.dt.float32, kind="ExternalOutput"
    )
    o_st = nc.dram_tensor("o_st", [P, NT], mybir.dt.float32, kind="ExternalOutput")

    with TileContext(nc) as tc:
        with (
            tc.tile_pool(name="work", bufs=2) as pool,
            tc.tile_pool(name="stats", bufs=1) as spool,
            tc.tile_pool(name="ps", bufs=1, space="PSUM") as psum,
        ):
            s_t = spool.tile([P, NT], mybir.dt.float32)
            ones = spool.tile([P, P], mybir.dt.bfloat16, tag="ones")
            ps_pt_a = psum.tile([P, MMN], mybir.dt.float32, tag="ps_pt_a")
            ps_pt_b = psum.tile([P, MMN], mybir.dt.float32, tag="ps_pt_b")
            ps_ppt_a = psum.tile([P, MMN], mybir.dt.float32, tag="ps_ppt_a")
            ps_ppt_b = psum.tile([P, MMN], mybir.dt.float32, tag="ps_ppt_b")
            ps_pp_a = psum.tile([P, MMN], mybir.dt.float32, tag="ps_pp_a")
            ps_pp_b = psum.tile([P, MMN], mybir.dt.float32, tag="ps_pp_b")

            xts, tts = [], []
            for i in range(NT):
                xt = pool.tile([P, F], mybir.dt.float32, tag="xt", bufs=6)
                tt = pool.tile([P, F], mybir.dt.int32, tag="tt", bufs=6)
                nc.sync.dma_start(xt[:], x[:, i * F : (i + 1) * F])
                nc.sync.dma_start(tt[:], t[:, i * F : (i + 1) * F])
                xts.append(xt)
                tts.append(tt)
                if i == 0:
                    nc.any.memset(ones, 1.0)

            for i in range(NT):
                xt, tt = xts[i], tts[i]
                p_ = pool.tile([P, F], mybir.dt.bfloat16, tag="p", bufs=3)
                tb = pool.tile([P, F], mybir.dt.bfloat16, tag="tb", bufs=3)
                u = pool.tile([P, F], mybir.dt.bfloat16, tag="u")
                w = pool.tile([P, F], mybir.dt.bfloat16, tag="w")
                sq = pool.tile([P, F], mybir.dt.bfloat16, tag="sq")

                nc.scalar.activation(
                    p_[:], xt[:], mybir.ActivationFunctionType.Sigmoid
                )
                nc.scalar.activation(
                    tb[:],
                    tt[:],
                    mybir.ActivationFunctionType.Copy,
                    accum_out=s_t[:, i : i + 1],
                )
                if i < NT - 1:
                    # u = p·tb: all-bf16 runs in DVE 2x perf mode
                    nc.vector.tensor_tensor(
                        out=u[:], in0=p_[:], in1=tb[:], op=mybir.AluOpType.mult
                    )
                else:
                    # last tile: u = p·t from the int32 tile directly (1x,
                    # slower) so u doesn't wait on the S_t copy — takes the
                    # copy off the end-of-stream drain's critical path
                    nc.vector.tensor_tensor(
                        out=u[:], in0=p_[:], in1=tt[:], op=mybir.AluOpType.mult
                    )
                nc.vector.tensor_tensor(
                    out=w[:], in0=u[:], in1=p_[:], op=mybir.AluOpType.mult
                )
                nc.vector.tensor_tensor(
                    out=sq[:], in0=p_[:], in1=p_[:], op=mybir.AluOpType.mult
                )
                for s_, banks in (
                    (u, (ps_pt_a, ps_pt_b)),
                    (w, (ps_ppt_a, ps_ppt_b)),
                    (sq, (ps_pp_a, ps_pp_b)),
                ):
                    for j in range(NCH):
                        nc.tensor.matmul(
                            banks[j % 2][:],
                            ones[:],
                            s_[:, j * MMN : (j + 1) * MMN],
                            start=(i == 0 and j < 2),
                            stop=(i == NT - 1 and j >= NCH - 2),
                        )

            fin = spool.tile([1, 6 * MMN], mybir.dt.float32, tag="fin")
            for k, ps in enumerate(
                (ps_pt_a, ps_pt_b, ps_ppt_a, ps_ppt_b, ps_pp_a, ps_pp_b)
            ):
                dst = fin[:, k * MMN : (k + 1) * MMN]
                if k % 2 == 0:
                    nc.vector.tensor_copy(dst, ps[0:1, :])
                else:
                    nc.scalar.copy(dst, ps[0:1, :])
            nc.sync.dma_start(o_sums[:], fin[:])
            nc.sync.dma_start(o_st[:], s_t[:])
    nc.finalize()
    return nc


def _get_nc():
    global _nc_cache
    if _nc_cache is None:
        _nc_cache = _build_bass()
    return _nc_cache


def kernel(input, target, _trace=False):
    x = np.ascontiguousarray(np.asarray(input, dtype=np.float32)).reshape(
        N_CORES, P, FREE
    )
    t = np.ascontiguousarray(np.asarray(target, dtype=np.int32)).reshape(
        N_CORES, P, FREE
    )
    in_maps = [{"input": x[i], "target": t[i]} for i in range(N_CORES)]

    nc = _get_nc()
    res = run_bass_kernel_spmd(
        nc, in_maps, core_ids=list(range(N_CORES)), trace=_trace
    )
    kernel.last_results = res

    s_pt = s_ppt = s_pp = s_t = 0.0
    for r in res.results:
        sums = r["o_sums"].astype(np.float64)
        s_pt += float(sums[0, 0 : 2 * MMN].sum())
        s_ppt += float(sums[0, 2 * MMN : 4 * MMN].sum())
        s_pp += float(sums[0, 4 * MMN :].sum())
        s_t += float(r["o_st"].astype(np.float64).sum())

    c1 = float(s_t)
    c0 = float(TOTAL - s_t)
    w0 = 1.0 / (c0 + SMOOTH) ** 2
    w1 = 1.0 / (c1 + SMOOTH) ** 2
    intersection = w1 * s_pt
    denominator = w0 * (s_pp - s_ppt) + w1 * (s_ppt + c1)
    dice = 1.0 - (2.0 * intersection + SMOOTH) / (denominator + SMOOTH)
    return np.asarray(dice, dtype=np.float32)
